# revision 19
# baseline (speedup 1.0000x reference)
"""Trainium2 Bass kernel for BackendQueryPooling.

Math simplifications used (all exact in exact arithmetic):
  - The k-projection folds into the shared query:
        scores[b,l,h] = x[b,l] . qw[h],  qw[h] = (q_h @ wk_head_h) / sqrt(hd)
    (the bk contribution is constant per h and cancels in softmax).
  - v is never materialized:
        ctx[n,h,:] = (sum_l w[n,h,l] x[b,l]) @ wvT_head + bv_head
    since sum_l w = 1 after normalization.
  - Per-(n,h) softmax uses unnormalized exp (scores are O(0.2), no overflow)
    with a multiplicative 0/1 mask; normalization by the sum computed via an
    appended ones-column in the same matmul.
  - Backends with no tokens produce 0 exactly as the reference does (the
    reference falls back to the full mask but then zeroes by has_tokens).

Sharding: data-parallel over batch. 16 batches / 8 cores = 2 batches per core.
No collectives; outputs are disjoint.

Pipeline: per batch, L is split into pieces. PE runs scores two pieces ahead
of the y-accumulation so the cross-engine exp->mask-mult chain (ACT->DVE)
never stalls the y matmuls; exp/w are additionally produced in half-piece
tiles for finer dependencies.
"""

import os
import sys

sys.path.insert(0, "/opt/trn_rl_repo")

import numpy as np
import ml_dtypes

import concourse.bass as bass
import concourse.bacc as bacc
import concourse.tile as tile
from concourse import mybir
from concourse.bass_utils import run_bass_kernel_spmd

BF16 = ml_dtypes.bfloat16
FP8 = ml_dtypes.float8_e4m3
QW_SHIFT = 13  # qw values ~7e-4 underflow fp8; pre-scale by 2**13, undo in exp scale
F32 = np.float32

B, L, D = 16, 8192, 256
H, HD, NB = 8, 32, 8
NCORES = 8
BPC = B // NCORES          # batches per core
NCH = L // 128             # 64 l-chunks of 128
SCALE = 1.0 / np.sqrt(HD)
NH = NB * H                # 64 (n, h) pairs
JW = D + 1                 # 257: x chunk width incl. ones column

_CACHE = {}
LAST_RESULT = None


def _patched_act_tables():
    """Make the act-table chooser land Exp AND Ln in one set
    (natural_log_exp_and_others) instead of thrashing between
    exp_and_others and natural_log (1.28us reload per switch).
    Order/size of the dict is preserved so act_func_set_ids stay valid."""
    from concourse.hw_specs import get_activation_tables

    AF = mybir.ActivationFunctionType

    def patched(arch):
        t = {k: set(v) for k, v in get_activation_tables(arch).items()}
        for name in t:
            if name != "natural_log_exp_and_others":
                t[name].discard(AF.Exp)
                t[name].discard(AF.Ln)
        return t

    return patched


def _build_nc(zero_bv=True, zero_br=True, unit_ln=True):
    nc = bacc.Bacc("TRN2", target_bir_lowering=False)
    dt = mybir.dt

    PIECES = [16, 16, 8, 16, 8]   # big early pieces keep the DMA ring fed
    FP8_XE = 40                   # l-chunks >= this stream the value path in fp8
    # (pieces 3,4 = 24 chunks; measured end-to-end rel err 1.53e-2 vs 2e-2 gate)
    NP = len(PIECES)
    OFFS = [sum(PIECES[:k]) for k in range(NP + 1)]
    LOOK = 2                      # scores lookahead (pieces)

    xT_d = nc.dram_tensor("xT", [BPC, 2, 128, L], dt.float8e4, kind="ExternalInput")
    xe_d = nc.dram_tensor("xext", [BPC, 128, 40 * JW], dt.bfloat16, kind="ExternalInput")
    xe8_d = nc.dram_tensor("xe8", [BPC, 128, (NCH - 40) * JW], dt.float8e4, kind="ExternalInput")
    m_d = nc.dram_tensor("m", [BPC, 128, NCH * NB], dt.bfloat16, kind="ExternalInput")
    qwT_d = nc.dram_tensor("qwT", [2, 128, H], dt.float8e4, kind="ExternalInput")
    wvT_d = nc.dram_tensor("wvT", [2, 128, D], dt.bfloat16, kind="ExternalInput")
    woT_d = nc.dram_tensor("woT", [2, 128, D], dt.bfloat16, kind="ExternalInput")
    if unit_ln:
        lht_d = nc.dram_tensor("lht", [BPC, NB, 1], dt.float32, kind="ExternalInput")
    else:
        gf_d = nc.dram_tensor("gf", [BPC, NB, D], dt.float32, kind="ExternalInput")
        bf_d = nc.dram_tensor("bf", [BPC, NB, D], dt.float32, kind="ExternalInput")
    br_d = nc.dram_tensor("br", [NB, D], dt.float32, kind="ExternalInput")
    bv_d = nc.dram_tensor("bv", [128, 2], dt.float32, kind="ExternalInput")
    id_d = nc.dram_tensor("ident", [NH, NH], dt.bfloat16, kind="ExternalInput")
    out_d = nc.dram_tensor("out", [BPC, NB, D], dt.float32, kind="ExternalOutput")

    AF = mybir.ActivationFunctionType

    with tile.TileContext(nc) as tc:
        with (
            tc.tile_pool(name="consts", bufs=1) as consts,
            tc.tile_pool(name="big", bufs=2) as big,
            tc.tile_pool(name="work", bufs=2) as work,
            tc.tile_pool(name="psc", bufs=3, space="PSUM") as psc,
            tc.tile_pool(name="psy", bufs=2, space="PSUM") as psy,
            tc.tile_pool(name="pst", bufs=2, space="PSUM") as pst,
        ):
            # qwT is on the scores critical path: load first
            qwT_sb = consts.tile([128, 2, H], dt.float8e4)
            nc.scalar.dma_start(out=qwT_sb, in_=qwT_d[:].rearrange("c p h -> p c h"))

            def load_tail_consts():
                # needed only from the first batch's tail onward
                wvT_sb = consts.tile([128, 2, D], dt.bfloat16)
                nc.gpsimd.dma_start(out=wvT_sb, in_=wvT_d[:].rearrange("c p e -> p c e"))
                woT_sb = consts.tile([128, 2, D], dt.bfloat16)
                nc.gpsimd.dma_start(out=woT_sb, in_=woT_d[:].rearrange("c p e -> p c e"))
                br_sb = None
                if not zero_br:
                    br_sb = consts.tile([NB, D], dt.float32)
                    nc.gpsimd.dma_start(out=br_sb, in_=br_d[:])
                bv_sb = None
                if not zero_bv:
                    bv_sb = consts.tile([128, 2], dt.float32)
                    nc.gpsimd.dma_start(out=bv_sb, in_=bv_d[:])
                id_sb = consts.tile([NH, NH], dt.bfloat16)
                nc.gpsimd.dma_start(out=id_sb, in_=id_d[:])
                eps_sb = consts.tile([128, 1], dt.float32)
                nc.vector.memset(eps_sb, 1e-5)
                return wvT_sb, woT_sb, br_sb, bv_sb, id_sb, eps_sb

            tail_consts = None
            for i in range(BPC):
                # ---- loads, interleaved in consumption order ----
                xT_sb, xe_sb = {}, {}
                m_sb = None

                def load_xe(qe):
                    # one DMA per piece (ring throughput is sensitive to DMA
                    # count); chunks >= FP8_XE come from the fp8 copy
                    pce = PIECES[qe]
                    if OFFS[qe] >= FP8_XE:
                        xe = big.tile([128, pce, JW], dt.float8e4, tag=f"xe{qe}_0")
                        o0 = OFFS[qe] - FP8_XE
                        nc.sync.dma_start(
                            out=xe,
                            in_=xe8_d[
                                i, :, o0 * JW:(o0 + pce) * JW
                            ].rearrange("p (c j) -> p c j", j=JW),
                        )
                    else:
                        xe = big.tile([128, pce, JW], dt.bfloat16, tag=f"xe{qe}_0")
                        nc.sync.dma_start(
                            out=xe,
                            in_=xe_d[
                                i, :, OFFS[qe] * JW:OFFS[qe + 1] * JW
                            ].rearrange("p (c j) -> p c j", j=JW),
                        )
                    xe_sb[(qe, 0)] = xe
                ht_sb = g_sb = b_sb = None
                # xT pieces 0..LOOK first (scores critical path + lookahead)
                for q in range(NP):
                    pc = PIECES[q]
                    t = big.tile([128, 2, pc * 128], dt.float8e4, tag=f"xT{q}")
                    nc.sync.dma_start(
                        out=t,
                        in_=xT_d[i, :, :, OFFS[q] * 128:OFFS[q + 1] * 128].rearrange(
                            "c p l -> p c l"
                        ),
                    )
                    xT_sb[q] = t
                    if q == 0:
                        m_sb = work.tile([128, NCH, NB], dt.bfloat16, tag="m")
                        nc.scalar.dma_start(
                            out=m_sb, in_=m_d[i].rearrange("p (c n) -> p c n", n=NB)
                        )
                    if q >= LOOK:
                        qe = q - LOOK
                        load_xe(qe)
                    if i == 0 and q == LOOK:
                        tail_consts = load_tail_consts()
                for qe in range(NP - LOOK, NP):
                    load_xe(qe)
                if unit_ln:
                    lht_sb = work.tile([NB, 1], dt.float32, tag="lht")
                    nc.gpsimd.dma_start(out=lht_sb, in_=lht_d[i])
                else:
                    g_sb = work.tile([NB, D], dt.float32, tag="g")
                    nc.gpsimd.dma_start(out=g_sb, in_=gf_d[i])
                    b_sb = work.tile([NB, D], dt.float32, tag="b")
                    nc.gpsimd.dma_start(out=b_sb, in_=bf_d[i])
                wvT_sb, woT_sb, br_sb, bv_sb, id_sb, eps_sb = tail_consts

                # ---- pipelined pieces: scores -> exp/w (half tiles) -> y ----
                y_ps = psy.tile([NH, JW], dt.float32, tag="y")

                def scores_piece(q):
                    pc = PIECES[q]
                    sc_ps = psc.tile([128, pc * H], dt.float32, tag="sc")
                    for cl in range(pc):
                        for dc in range(2):
                            nc.tensor.matmul(
                                sc_ps[:, cl * 8:(cl + 1) * 8],
                                lhsT=xT_sb[q][:, dc, cl * 128:(cl + 1) * 128],
                                rhs=qwT_sb[:, dc, :],
                                start=(dc == 0),
                                stop=(dc == 1),
                            )
                    return sc_ps

                def expw_piece(q, sc_ps):
                    pc = PIECES[q]
                    h1 = pc // 2
                    ws = []
                    for part, (c0, c1) in enumerate([(0, h1), (h1, pc)]):
                        seg = c1 - c0
                        E = work.tile([128, seg, H], dt.bfloat16, tag=f"E{q}_{part}")
                        nc.scalar.activation(
                            out=E.rearrange("p c h -> p (c h)"),
                            in_=sc_ps[:, c0 * 8:c1 * 8],
                            func=AF.Exp,
                            scale=float(2.0 ** -QW_SHIFT),
                        )
                        w = big.tile([128, seg, NB, H], dt.bfloat16, tag=f"w{q}_{part}")
                        nc.vector.tensor_tensor(
                            out=w,
                            in0=E.unsqueeze(2).broadcast_to([128, seg, NB, H]),
                            in1=m_sb[:, OFFS[q] + c0:OFFS[q] + c1].unsqueeze(3).broadcast_to(
                                [128, seg, NB, H]
                            ),
                            op=mybir.AluOpType.mult,
                        )
                        ws.append((c0, seg, w))
                    return ws

                def y_piece(q, ws):
                    for c0, seg, w in ws:
                        for k in range(seg):
                            cl = c0 + k
                            nc.tensor.matmul(
                                y_ps,
                                lhsT=w[:, k],
                                rhs=xe_sb[(q, 0)][:, cl],
                                start=(q == 0 and cl == 0),
                                stop=(q == NP - 1 and cl == PIECES[q] - 1),
                            )

                wq = {}
                for q in range(NP):
                    wq[q] = expw_piece(q, scores_piece(q))
                    if q >= LOOK:
                        y_piece(q - LOOK, wq.pop(q - LOOK))
                for q in range(NP - LOOK, NP):
                    y_piece(q, wq.pop(q))

                # ---- normalize ----
                # host builds m with the reference's full-mask fallback for
                # empty backends, so s = sum(w) > 0 always: no epsilon needed
                r_sb = work.tile([NH, 1], dt.float32, tag="r")
                nc.vector.reciprocal(r_sb, y_ps[:, D:JW])
                yn_sb = work.tile([NH, D], dt.bfloat16, tag="yn")
                nc.vector.tensor_scalar_mul(yn_sb, y_ps[:, 0:D], r_sb)

                # ---- transpose y_norm -> yT [d, nh] (2 chunks) ----
                yT_sb = work.tile([128, 2, NH], dt.bfloat16, tag="yT")
                for ec in range(2):
                    tr_ps = pst.tile([128, NH], dt.bfloat16, tag="tail")
                    nc.tensor.transpose(
                        tr_ps, yn_sb[:, ec * 128:(ec + 1) * 128], id_sb
                    )
                    nc.vector.tensor_copy(yT_sb[:, ec, :], tr_ps)

                # ---- ctxT[e', (t, n)] via per-head matmuls ----
                cx_ps = pst.tile([128, 2 * NB], dt.float32, tag="tail")
                for h in range(H):
                    t, r4 = divmod(h, 4)
                    r0 = r4 * 32
                    for dc in range(2):
                        nc.tensor.matmul(
                            cx_ps[r0:r0 + 32, t * NB:(t + 1) * NB],
                            lhsT=wvT_sb[:, dc, 32 * h:32 * h + 32],
                            rhs=yT_sb[:, dc, :].rearrange("p (n h2) -> p n h2", h2=H)[:, :, h],
                            start=(dc == 0),
                            stop=(dc == 1),
                            tile_position=(0, r0),
                        )
                cx_sb = work.tile([128, 2 * NB], dt.bfloat16, tag="cxs")
                nc.vector.tensor_copy(cx_sb, cx_ps)
                if not zero_bv:
                    for t in range(2):
                        nc.vector.tensor_scalar_add(
                            cx_sb[:, t * NB:(t + 1) * NB],
                            cx_sb[:, t * NB:(t + 1) * NB],
                            bv_sb[:, t:t + 1],
                        )

                # ---- out_proj + bias + LayerNorm (folded has_tokens) ----
                op_ps = pst.tile([NB, D], dt.float32, tag="tail")
                for t in range(2):
                    nc.tensor.matmul(
                        op_ps,
                        lhsT=cx_sb[:, t * NB:(t + 1) * NB],
                        rhs=woT_sb[:, t, :],
                        start=(t == 0),
                        stop=(t == 1),
                    )
                if zero_br:
                    o_ap = op_ps
                else:
                    o_sb = work.tile([NB, D], dt.float32, tag="o")
                    nc.vector.tensor_tensor(o_sb, op_ps, br_sb, op=mybir.AluOpType.add)
                    o_ap = o_sb
                st_sb = work.tile([NB, 6], dt.float32, tag="st")
                nc.vector.bn_stats(st_sb, o_ap)
                mv_sb = work.tile([NB, 2], dt.float32, tag="mv")
                nc.vector.bn_aggr(mv_sb, st_sb)
                lnv_sb = work.tile([NB, 1], dt.float32, tag="lnv")
                nc.scalar.activation(
                    lnv_sb, mv_sb[:, 1:2], func=AF.Ln,
                    bias=eps_sb[0:NB],
                )
                rstd_sb = work.tile([NB, 1], dt.float32, tag="rstd")
                if unit_ln:
                    # rstd = exp(-0.5*ln(var+eps) + log(has_tokens)): the
                    # -1e30 bias for empty backends drives rstd to exactly 0,
                    # folding the has_tokens gate into this activation
                    nc.scalar.activation(
                        rstd_sb, lnv_sb, func=AF.Exp,
                        scale=-0.5, bias=lht_sb,
                    )
                else:
                    nc.scalar.activation(
                        rstd_sb, lnv_sb, func=AF.Exp,
                        scale=-0.5,
                    )
                c_sb = work.tile([NB, D], dt.float32, tag="c")
                if unit_ln:
                    nc.vector.tensor_scalar(
                        c_sb, o_ap, mv_sb[:, 0:1], rstd_sb,
                        op0=mybir.AluOpType.subtract, op1=mybir.AluOpType.mult,
                    )
                else:
                    nc.vector.tensor_scalar(
                        c_sb, o_ap, mv_sb[:, 0:1], rstd_sb,
                        op0=mybir.AluOpType.subtract, op1=mybir.AluOpType.mult,
                    )
                    nc.vector.tensor_tensor(c_sb, c_sb, g_sb, op=mybir.AluOpType.mult)
                    nc.vector.tensor_tensor(c_sb, c_sb, b_sb, op=mybir.AluOpType.add)

                # SWDGE store for batch 0 keeps the result-gated store off the
                # Sync HWDGE ring (later loads would queue behind it); the final
                # batch has no later loads, so use the faster HWDGE path.
                if i == BPC - 1:
                    nc.sync.dma_start(out=out_d[i], in_=c_sb)
                else:
                    nc.gpsimd.dma_start(out=out_d[i], in_=c_sb)

    import concourse.bacc as bacc_mod

    orig_tables = bacc_mod.get_activation_tables
    bacc_mod.get_activation_tables = _patched_act_tables()
    try:
        nc.compile()
    finally:
        bacc_mod.get_activation_tables = orig_tables
    return nc


def _get_nc(zero_bv=True, zero_br=True, unit_ln=True):
    key = ("nc", zero_bv, zero_br, unit_ln)
    if key not in _CACHE:
        _CACHE[key] = _build_nc(zero_bv, zero_br, unit_ln)
    return _CACHE[key]


def _prep(inputs, unit_ln):
    x = np.asarray(inputs["x"], F32)
    query = np.asarray(inputs["query"], F32)
    ipw = np.asarray(inputs["in_proj_weight"], F32)
    ipb = np.asarray(inputs["in_proj_bias"], F32)
    opw = np.asarray(inputs["out_proj_weight"], F32)
    opb = np.asarray(inputs["out_proj_bias"], F32)
    gamma = np.asarray(inputs["ln_gamma"], F32)
    beta = np.asarray(inputs["ln_beta"], F32)
    mask = np.asarray(inputs["mask"]).astype(bool)
    bid = np.asarray(inputs["backend_id"]).astype(np.int32)
    nbm = int(np.asarray(inputs["n_backends_max"]))
    assert nbm == NB and x.shape == (B, L, D)

    wq, wk, wv = ipw[0:D], ipw[D:2 * D], ipw[2 * D:3 * D]
    bq, bk, bv = ipb[0:D], ipb[D:2 * D], ipb[2 * D:3 * D]

    qv = query[0, 0] @ wq.T + bq                      # (256,)
    qh = qv.reshape(H, HD)
    qw = np.einsum("hj,hjd->hd", qh, wk.reshape(H, HD, D)) * SCALE  # (8, 256)
    # bk contribution is constant per h -> cancels in softmax normalization.

    qwT = np.ascontiguousarray(qw.T * 2.0 ** QW_SHIFT).reshape(2, 128, H).astype(FP8)
    wvT = np.ascontiguousarray(wv.T).reshape(2, 128, D).astype(BF16)
    woT = np.ascontiguousarray(opw.T).reshape(2, 128, D).astype(BF16)
    bv_dev = np.ascontiguousarray(bv.reshape(2, 128).T).astype(F32)   # [e', t]
    br = np.ascontiguousarray(np.broadcast_to(opb, (NB, D))).astype(F32)
    ident = np.eye(NH, dtype=BF16)

    bm = mask[:, :, None] & (bid[:, :, None] == np.arange(NB, dtype=np.int32))
    ht_b = bm.any(1)                                  # (16, 8) bool
    ht = ht_b.astype(F32)
    # reference's fallback: empty backends use the full mask (their output is
    # zeroed by has_tokens anyway); guarantees sum(w) > 0 on device. If even
    # the full mask is empty, use all-ones (still zeroed by ht).
    fall = np.where(mask.any(axis=1)[:, None], mask, True)
    bm_safe = np.where(ht_b[:, None, :], bm, fall[:, :, None])
    m_host = np.ascontiguousarray(
        bm_safe.reshape(B, NCH, 128, NB).transpose(0, 2, 1, 3)
    ).astype(BF16).reshape(B, 128, NCH * NB)

    x_bf = x.astype(BF16)
    xT = np.ascontiguousarray(x.transpose(0, 2, 1)).reshape(B, 2, 128, L).astype(FP8)
    x_pcd = x_bf.reshape(B, NCH, 128, D).transpose(0, 2, 1, 3)  # [B,128,NCH,D]
    FP8_XE = 40
    xe = np.empty((B, 128, FP8_XE, JW), BF16)
    xe[..., :D] = x_pcd[:, :, :FP8_XE]
    xe[..., D] = 1
    xe = xe.reshape(B, 128, FP8_XE * JW)
    xe8 = np.empty((B, 128, NCH - FP8_XE, JW), FP8)
    xe8[..., :D] = x_pcd[:, :, FP8_XE:]
    xe8[..., D] = 1
    xe8 = xe8.reshape(B, 128, (NCH - FP8_XE) * JW)

    in_maps = []
    for c in range(NCORES):
        sl = slice(BPC * c, BPC * (c + 1))
        im = {
            "xT": xT[sl], "xext": xe[sl], "xe8": xe8[sl], "m": m_host[sl],
            "qwT": qwT, "wvT": wvT, "woT": woT,
            "br": br, "bv": bv_dev,
            "ident": ident,
        }
        if unit_ln:
            lht = np.where(ht[sl] > 0, 0.0, -1e30).astype(F32)
            im["lht"] = np.ascontiguousarray(lht[:, :, None])
        else:
            gf = np.ascontiguousarray(gamma[None, None, :] * ht[sl, :, None]).astype(F32)
            bf = np.ascontiguousarray(beta[None, None, :] * ht[sl, :, None]).astype(F32)
            im["gf"] = gf
            im["bf"] = bf
        in_maps.append(im)
    return in_maps


def kernel(**inputs):
    global LAST_RESULT
    gamma = np.asarray(inputs["ln_gamma"], F32)
    beta = np.asarray(inputs["ln_beta"], F32)
    unit_ln = bool((gamma == 1.0).all() and not beta.any())
    in_maps = _prep(inputs, unit_ln)
    ipb = np.asarray(inputs["in_proj_bias"], F32)
    opb = np.asarray(inputs["out_proj_bias"], F32)
    nc = _get_nc(zero_bv=not ipb[2 * D:].any(), zero_br=not opb.any(),
                 unit_ln=unit_ln)
    res = run_bass_kernel_spmd(nc, in_maps, list(range(NCORES)))
    LAST_RESULT = res
    out = np.concatenate([res.results[c]["out"] for c in range(NCORES)], axis=0)
    return np.ascontiguousarray(out.astype(F32))


if __name__ == "__main__":
    nc = _get_nc()
    print("traced ok:", nc)


# revision 20
# speedup vs baseline: 1.0323x; 1.0323x over previous
"""Trainium2 Bass kernel for BackendQueryPooling.

Math simplifications used (all exact in exact arithmetic):
  - The k-projection folds into the shared query:
        scores[b,l,h] = x[b,l] . qw[h],  qw[h] = (q_h @ wk_head_h) / sqrt(hd)
    (the bk contribution is constant per h and cancels in softmax).
  - v is never materialized:
        ctx[n,h,:] = (sum_l w[n,h,l] x[b,l]) @ wvT_head + bv_head
    since sum_l w = 1 after normalization.
  - Per-(n,h) softmax uses unnormalized exp (scores are O(0.2), no overflow)
    with a multiplicative 0/1 mask; normalization by the sum computed via an
    appended ones-column in the same matmul.
  - Backends with no tokens produce 0 exactly as the reference does (the
    reference falls back to the full mask but then zeroes by has_tokens).

Sharding: data-parallel over batch. 16 batches / 8 cores = 2 batches per core.
No collectives; outputs are disjoint.

Pipeline: per batch, L is split into pieces. PE runs scores two pieces ahead
of the y-accumulation so the cross-engine exp->mask-mult chain (ACT->DVE)
never stalls the y matmuls; exp/w are additionally produced in half-piece
tiles for finer dependencies.
"""

import os
import sys

sys.path.insert(0, "/opt/trn_rl_repo")

import numpy as np
import ml_dtypes

import concourse.bass as bass
import concourse.bacc as bacc
import concourse.tile as tile
from concourse import mybir
from concourse.bass_utils import run_bass_kernel_spmd

BF16 = ml_dtypes.bfloat16
FP8 = ml_dtypes.float8_e4m3
QW_SHIFT = 13  # qw values ~7e-4 underflow fp8; pre-scale by 2**13, undo in exp scale
F32 = np.float32

B, L, D = 16, 8192, 256
H, HD, NB = 8, 32, 8
NCORES = 8
BPC = B // NCORES          # batches per core
NCH = L // 128             # 64 l-chunks of 128
SCALE = 1.0 / np.sqrt(HD)
NH = NB * H                # 64 (n, h) pairs
JW = D + 1                 # 257: x chunk width incl. ones column

_CACHE = {}
LAST_RESULT = None


def _patched_act_tables():
    """Make the act-table chooser land Exp AND Ln in one set
    (natural_log_exp_and_others) instead of thrashing between
    exp_and_others and natural_log (1.28us reload per switch).
    Order/size of the dict is preserved so act_func_set_ids stay valid."""
    from concourse.hw_specs import get_activation_tables

    AF = mybir.ActivationFunctionType

    def patched(arch):
        t = {k: set(v) for k, v in get_activation_tables(arch).items()}
        for name in t:
            if name != "natural_log_exp_and_others":
                t[name].discard(AF.Exp)
                t[name].discard(AF.Ln)
        return t

    return patched


def _build_nc(zero_bv=True, zero_br=True, unit_ln=True):
    nc = bacc.Bacc("TRN2", target_bir_lowering=False)
    dt = mybir.dt

    PIECES = [16, 16, 8, 16, 8]   # big early pieces keep the DMA ring fed
    # whole value path in fp8 e3m4 (4 mantissa bits): sim rel err 1.35e-2 vs
    # the 2e-2 gate, and the xe stream halves vs bf16
    NP = len(PIECES)
    OFFS = [sum(PIECES[:k]) for k in range(NP + 1)]
    LOOK = 2                      # scores lookahead (pieces)

    xT_d = nc.dram_tensor("xT", [BPC, 2, 128, L], dt.float8e4, kind="ExternalInput")
    xe8_d = nc.dram_tensor("xe8", [BPC, 128, NCH * JW], dt.float8e3, kind="ExternalInput")
    m_d = nc.dram_tensor("m", [BPC, 128, NCH * NB], dt.bfloat16, kind="ExternalInput")
    qwT_d = nc.dram_tensor("qwT", [2, 128, H], dt.float8e4, kind="ExternalInput")
    wvT_d = nc.dram_tensor("wvT", [2, 128, D], dt.bfloat16, kind="ExternalInput")
    woT_d = nc.dram_tensor("woT", [2, 128, D], dt.bfloat16, kind="ExternalInput")
    if unit_ln:
        lht_d = nc.dram_tensor("lht", [BPC, NB, 1], dt.float32, kind="ExternalInput")
    else:
        gf_d = nc.dram_tensor("gf", [BPC, NB, D], dt.float32, kind="ExternalInput")
        bf_d = nc.dram_tensor("bf", [BPC, NB, D], dt.float32, kind="ExternalInput")
    br_d = nc.dram_tensor("br", [NB, D], dt.float32, kind="ExternalInput")
    bv_d = nc.dram_tensor("bv", [128, 2], dt.float32, kind="ExternalInput")
    id_d = nc.dram_tensor("ident", [NH, NH], dt.bfloat16, kind="ExternalInput")
    out_d = nc.dram_tensor("out", [BPC, NB, D], dt.float32, kind="ExternalOutput")

    AF = mybir.ActivationFunctionType

    with tile.TileContext(nc) as tc:
        with (
            tc.tile_pool(name="consts", bufs=1) as consts,
            tc.tile_pool(name="big", bufs=2) as big,
            tc.tile_pool(name="work", bufs=2) as work,
            tc.tile_pool(name="psc", bufs=3, space="PSUM") as psc,
            tc.tile_pool(name="psy", bufs=2, space="PSUM") as psy,
            tc.tile_pool(name="pst", bufs=2, space="PSUM") as pst,
        ):
            # qwT is on the scores critical path: load first
            qwT_sb = consts.tile([128, 2, H], dt.float8e4)
            nc.scalar.dma_start(out=qwT_sb, in_=qwT_d[:].rearrange("c p h -> p c h"))

            def load_tail_consts():
                # needed only from the first batch's tail onward
                wvT_sb = consts.tile([128, 2, D], dt.bfloat16)
                nc.gpsimd.dma_start(out=wvT_sb, in_=wvT_d[:].rearrange("c p e -> p c e"))
                woT_sb = consts.tile([128, 2, D], dt.bfloat16)
                nc.gpsimd.dma_start(out=woT_sb, in_=woT_d[:].rearrange("c p e -> p c e"))
                br_sb = None
                if not zero_br:
                    br_sb = consts.tile([NB, D], dt.float32)
                    nc.gpsimd.dma_start(out=br_sb, in_=br_d[:])
                bv_sb = None
                if not zero_bv:
                    bv_sb = consts.tile([128, 2], dt.float32)
                    nc.gpsimd.dma_start(out=bv_sb, in_=bv_d[:])
                id_sb = consts.tile([NH, NH], dt.bfloat16)
                nc.gpsimd.dma_start(out=id_sb, in_=id_d[:])
                eps_sb = consts.tile([128, 1], dt.float32)
                nc.vector.memset(eps_sb, 1e-5)
                return wvT_sb, woT_sb, br_sb, bv_sb, id_sb, eps_sb

            tail_consts = None
            for i in range(BPC):
                # ---- loads, interleaved in consumption order ----
                xT_sb, xe_sb = {}, {}
                m_sb = None

                def load_xe(qe):
                    # one DMA per piece (ring throughput is sensitive to DMA
                    # count); value path is e3m4 end to end
                    pce = PIECES[qe]
                    xe = big.tile([128, pce, JW], dt.float8e3, tag=f"xe{qe}_0")
                    nc.sync.dma_start(
                        out=xe,
                        in_=xe8_d[
                            i, :, OFFS[qe] * JW:OFFS[qe + 1] * JW
                        ].rearrange("p (c j) -> p c j", j=JW),
                    )
                    xe_sb[(qe, 0)] = xe
                ht_sb = g_sb = b_sb = None
                # xT pieces 0..LOOK first (scores critical path + lookahead)
                for q in range(NP):
                    pc = PIECES[q]
                    t = big.tile([128, 2, pc * 128], dt.float8e4, tag=f"xT{q}")
                    nc.sync.dma_start(
                        out=t,
                        in_=xT_d[i, :, :, OFFS[q] * 128:OFFS[q + 1] * 128].rearrange(
                            "c p l -> p c l"
                        ),
                    )
                    xT_sb[q] = t
                    if q == 0:
                        m_sb = work.tile([128, NCH, NB], dt.bfloat16, tag="m")
                        nc.scalar.dma_start(
                            out=m_sb, in_=m_d[i].rearrange("p (c n) -> p c n", n=NB)
                        )
                    if q >= LOOK:
                        qe = q - LOOK
                        load_xe(qe)
                    if i == 0 and q == LOOK:
                        tail_consts = load_tail_consts()
                for qe in range(NP - LOOK, NP):
                    load_xe(qe)
                if unit_ln:
                    lht_sb = work.tile([NB, 1], dt.float32, tag="lht")
                    nc.gpsimd.dma_start(out=lht_sb, in_=lht_d[i])
                else:
                    g_sb = work.tile([NB, D], dt.float32, tag="g")
                    nc.gpsimd.dma_start(out=g_sb, in_=gf_d[i])
                    b_sb = work.tile([NB, D], dt.float32, tag="b")
                    nc.gpsimd.dma_start(out=b_sb, in_=bf_d[i])
                wvT_sb, woT_sb, br_sb, bv_sb, id_sb, eps_sb = tail_consts

                # ---- pipelined pieces: scores -> exp/w (half tiles) -> y ----
                y_ps = psy.tile([NH, JW], dt.float32, tag="y")

                def scores_piece(q):
                    pc = PIECES[q]
                    sc_ps = psc.tile([128, pc * H], dt.float32, tag="sc")
                    for cl in range(pc):
                        for dc in range(2):
                            nc.tensor.matmul(
                                sc_ps[:, cl * 8:(cl + 1) * 8],
                                lhsT=xT_sb[q][:, dc, cl * 128:(cl + 1) * 128],
                                rhs=qwT_sb[:, dc, :],
                                start=(dc == 0),
                                stop=(dc == 1),
                            )
                    return sc_ps

                def expw_piece(q, sc_ps):
                    pc = PIECES[q]
                    h1 = pc // 2
                    ws = []
                    for part, (c0, c1) in enumerate([(0, h1), (h1, pc)]):
                        seg = c1 - c0
                        E = work.tile([128, seg, H], dt.bfloat16, tag=f"E{q}_{part}")
                        nc.scalar.activation(
                            out=E.rearrange("p c h -> p (c h)"),
                            in_=sc_ps[:, c0 * 8:c1 * 8],
                            func=AF.Exp,
                            scale=float(2.0 ** -QW_SHIFT),
                        )
                        w = big.tile([128, seg, NB, H], dt.bfloat16, tag=f"w{q}_{part}")
                        nc.vector.tensor_tensor(
                            out=w,
                            in0=E.unsqueeze(2).broadcast_to([128, seg, NB, H]),
                            in1=m_sb[:, OFFS[q] + c0:OFFS[q] + c1].unsqueeze(3).broadcast_to(
                                [128, seg, NB, H]
                            ),
                            op=mybir.AluOpType.mult,
                        )
                        ws.append((c0, seg, w))
                    return ws

                def y_piece(q, ws):
                    for c0, seg, w in ws:
                        for k in range(seg):
                            cl = c0 + k
                            nc.tensor.matmul(
                                y_ps,
                                lhsT=w[:, k],
                                rhs=xe_sb[(q, 0)][:, cl],
                                start=(q == 0 and cl == 0),
                                stop=(q == NP - 1 and cl == PIECES[q] - 1),
                            )

                wq = {}
                for q in range(NP):
                    wq[q] = expw_piece(q, scores_piece(q))
                    if q >= LOOK:
                        y_piece(q - LOOK, wq.pop(q - LOOK))
                for q in range(NP - LOOK, NP):
                    y_piece(q, wq.pop(q))

                # ---- normalize ----
                # host builds m with the reference's full-mask fallback for
                # empty backends, so s = sum(w) > 0 always: no epsilon needed
                r_sb = work.tile([NH, 1], dt.float32, tag="r")
                nc.vector.reciprocal(r_sb, y_ps[:, D:JW])
                yn_sb = work.tile([NH, D], dt.bfloat16, tag="yn")
                nc.vector.tensor_scalar_mul(yn_sb, y_ps[:, 0:D], r_sb)

                # ---- transpose y_norm -> yT [d, nh] (2 chunks) ----
                yT_sb = work.tile([128, 2, NH], dt.bfloat16, tag="yT")
                for ec in range(2):
                    tr_ps = pst.tile([128, NH], dt.bfloat16, tag="tail")
                    nc.tensor.transpose(
                        tr_ps, yn_sb[:, ec * 128:(ec + 1) * 128], id_sb
                    )
                    nc.vector.tensor_copy(yT_sb[:, ec, :], tr_ps)

                # ---- ctxT[e', (t, n)] via per-head matmuls ----
                cx_ps = pst.tile([128, 2 * NB], dt.float32, tag="tail")
                for h in range(H):
                    t, r4 = divmod(h, 4)
                    r0 = r4 * 32
                    for dc in range(2):
                        nc.tensor.matmul(
                            cx_ps[r0:r0 + 32, t * NB:(t + 1) * NB],
                            lhsT=wvT_sb[:, dc, 32 * h:32 * h + 32],
                            rhs=yT_sb[:, dc, :].rearrange("p (n h2) -> p n h2", h2=H)[:, :, h],
                            start=(dc == 0),
                            stop=(dc == 1),
                            tile_position=(0, r0),
                        )
                cx_sb = work.tile([128, 2 * NB], dt.bfloat16, tag="cxs")
                nc.vector.tensor_copy(cx_sb, cx_ps)
                if not zero_bv:
                    for t in range(2):
                        nc.vector.tensor_scalar_add(
                            cx_sb[:, t * NB:(t + 1) * NB],
                            cx_sb[:, t * NB:(t + 1) * NB],
                            bv_sb[:, t:t + 1],
                        )

                # ---- out_proj + bias + LayerNorm (folded has_tokens) ----
                op_ps = pst.tile([NB, D], dt.float32, tag="tail")
                for t in range(2):
                    nc.tensor.matmul(
                        op_ps,
                        lhsT=cx_sb[:, t * NB:(t + 1) * NB],
                        rhs=woT_sb[:, t, :],
                        start=(t == 0),
                        stop=(t == 1),
                    )
                if zero_br:
                    o_ap = op_ps
                else:
                    o_sb = work.tile([NB, D], dt.float32, tag="o")
                    nc.vector.tensor_tensor(o_sb, op_ps, br_sb, op=mybir.AluOpType.add)
                    o_ap = o_sb
                st_sb = work.tile([NB, 6], dt.float32, tag="st")
                nc.vector.bn_stats(st_sb, o_ap)
                mv_sb = work.tile([NB, 2], dt.float32, tag="mv")
                nc.vector.bn_aggr(mv_sb, st_sb)
                lnv_sb = work.tile([NB, 1], dt.float32, tag="lnv")
                nc.scalar.activation(
                    lnv_sb, mv_sb[:, 1:2], func=AF.Ln,
                    bias=eps_sb[0:NB],
                )
                rstd_sb = work.tile([NB, 1], dt.float32, tag="rstd")
                if unit_ln:
                    # rstd = exp(-0.5*ln(var+eps) + log(has_tokens)): the
                    # -1e30 bias for empty backends drives rstd to exactly 0,
                    # folding the has_tokens gate into this activation
                    nc.scalar.activation(
                        rstd_sb, lnv_sb, func=AF.Exp,
                        scale=-0.5, bias=lht_sb,
                    )
                else:
                    nc.scalar.activation(
                        rstd_sb, lnv_sb, func=AF.Exp,
                        scale=-0.5,
                    )
                c_sb = work.tile([NB, D], dt.float32, tag="c")
                if unit_ln:
                    nc.vector.tensor_scalar(
                        c_sb, o_ap, mv_sb[:, 0:1], rstd_sb,
                        op0=mybir.AluOpType.subtract, op1=mybir.AluOpType.mult,
                    )
                else:
                    nc.vector.tensor_scalar(
                        c_sb, o_ap, mv_sb[:, 0:1], rstd_sb,
                        op0=mybir.AluOpType.subtract, op1=mybir.AluOpType.mult,
                    )
                    nc.vector.tensor_tensor(c_sb, c_sb, g_sb, op=mybir.AluOpType.mult)
                    nc.vector.tensor_tensor(c_sb, c_sb, b_sb, op=mybir.AluOpType.add)

                # SWDGE store for batch 0 keeps the result-gated store off the
                # Sync HWDGE ring (later loads would queue behind it); the final
                # batch has no later loads, so use the faster HWDGE path.
                if i == BPC - 1:
                    nc.sync.dma_start(out=out_d[i], in_=c_sb)
                else:
                    nc.gpsimd.dma_start(out=out_d[i], in_=c_sb)

    import concourse.bacc as bacc_mod

    orig_tables = bacc_mod.get_activation_tables
    bacc_mod.get_activation_tables = _patched_act_tables()
    try:
        nc.compile()
    finally:
        bacc_mod.get_activation_tables = orig_tables
    return nc


def _get_nc(zero_bv=True, zero_br=True, unit_ln=True):
    key = ("nc", zero_bv, zero_br, unit_ln)
    if key not in _CACHE:
        _CACHE[key] = _build_nc(zero_bv, zero_br, unit_ln)
    return _CACHE[key]


def _prep(inputs, unit_ln):
    x = np.asarray(inputs["x"], F32)
    query = np.asarray(inputs["query"], F32)
    ipw = np.asarray(inputs["in_proj_weight"], F32)
    ipb = np.asarray(inputs["in_proj_bias"], F32)
    opw = np.asarray(inputs["out_proj_weight"], F32)
    opb = np.asarray(inputs["out_proj_bias"], F32)
    gamma = np.asarray(inputs["ln_gamma"], F32)
    beta = np.asarray(inputs["ln_beta"], F32)
    mask = np.asarray(inputs["mask"]).astype(bool)
    bid = np.asarray(inputs["backend_id"]).astype(np.int32)
    nbm = int(np.asarray(inputs["n_backends_max"]))
    assert nbm == NB and x.shape == (B, L, D)

    wq, wk, wv = ipw[0:D], ipw[D:2 * D], ipw[2 * D:3 * D]
    bq, bk, bv = ipb[0:D], ipb[D:2 * D], ipb[2 * D:3 * D]

    qv = query[0, 0] @ wq.T + bq                      # (256,)
    qh = qv.reshape(H, HD)
    qw = np.einsum("hj,hjd->hd", qh, wk.reshape(H, HD, D)) * SCALE  # (8, 256)
    # bk contribution is constant per h -> cancels in softmax normalization.

    qwT = np.ascontiguousarray(qw.T * 2.0 ** QW_SHIFT).reshape(2, 128, H).astype(FP8)
    wvT = np.ascontiguousarray(wv.T).reshape(2, 128, D).astype(BF16)
    woT = np.ascontiguousarray(opw.T).reshape(2, 128, D).astype(BF16)
    bv_dev = np.ascontiguousarray(bv.reshape(2, 128).T).astype(F32)   # [e', t]
    br = np.ascontiguousarray(np.broadcast_to(opb, (NB, D))).astype(F32)
    ident = np.eye(NH, dtype=BF16)

    bm = mask[:, :, None] & (bid[:, :, None] == np.arange(NB, dtype=np.int32))
    ht_b = bm.any(1)                                  # (16, 8) bool
    ht = ht_b.astype(F32)
    # reference's fallback: empty backends use the full mask (their output is
    # zeroed by has_tokens anyway); guarantees sum(w) > 0 on device. If even
    # the full mask is empty, use all-ones (still zeroed by ht).
    fall = np.where(mask.any(axis=1)[:, None], mask, True)
    bm_safe = np.where(ht_b[:, None, :], bm, fall[:, :, None])
    m_host = np.ascontiguousarray(
        bm_safe.reshape(B, NCH, 128, NB).transpose(0, 2, 1, 3)
    ).astype(BF16).reshape(B, 128, NCH * NB)

    x_bf = x.astype(BF16)
    xT = np.ascontiguousarray(x.transpose(0, 2, 1)).reshape(B, 2, 128, L).astype(FP8)
    x_pcd = x_bf.reshape(B, NCH, 128, D).transpose(0, 2, 1, 3)  # [B,128,NCH,D]
    E3 = ml_dtypes.float8_e3m4
    xe8 = np.empty((B, 128, NCH, JW), E3)
    xe8[..., :D] = x_pcd
    xe8[..., D] = 1
    xe8 = xe8.reshape(B, 128, NCH * JW)

    in_maps = []
    for c in range(NCORES):
        sl = slice(BPC * c, BPC * (c + 1))
        im = {
            "xT": xT[sl], "xe8": xe8[sl], "m": m_host[sl],
            "qwT": qwT, "wvT": wvT, "woT": woT,
            "br": br, "bv": bv_dev,
            "ident": ident,
        }
        if unit_ln:
            lht = np.where(ht[sl] > 0, 0.0, -1e30).astype(F32)
            im["lht"] = np.ascontiguousarray(lht[:, :, None])
        else:
            gf = np.ascontiguousarray(gamma[None, None, :] * ht[sl, :, None]).astype(F32)
            bf = np.ascontiguousarray(beta[None, None, :] * ht[sl, :, None]).astype(F32)
            im["gf"] = gf
            im["bf"] = bf
        in_maps.append(im)
    return in_maps


def kernel(**inputs):
    global LAST_RESULT
    gamma = np.asarray(inputs["ln_gamma"], F32)
    beta = np.asarray(inputs["ln_beta"], F32)
    unit_ln = bool((gamma == 1.0).all() and not beta.any())
    in_maps = _prep(inputs, unit_ln)
    ipb = np.asarray(inputs["in_proj_bias"], F32)
    opb = np.asarray(inputs["out_proj_bias"], F32)
    nc = _get_nc(zero_bv=not ipb[2 * D:].any(), zero_br=not opb.any(),
                 unit_ln=unit_ln)
    res = run_bass_kernel_spmd(nc, in_maps, list(range(NCORES)))
    LAST_RESULT = res
    out = np.concatenate([res.results[c]["out"] for c in range(NCORES)], axis=0)
    return np.ascontiguousarray(out.astype(F32))


if __name__ == "__main__":
    nc = _get_nc()
    print("traced ok:", nc)


# revision 24
# speedup vs baseline: 1.2793x; 1.2392x over previous
"""Trainium2 Bass kernel for BackendQueryPooling.

Math simplifications used (all exact in exact arithmetic):
  - The k-projection folds into the shared query:
        scores[b,l,h] = x[b,l] . qw[h],  qw[h] = (q_h @ wk_head_h) / sqrt(hd)
    (the bk contribution is constant per h and cancels in softmax).
  - v is never materialized:
        ctx[n,h,:] = (sum_l w[n,h,l] x[b,l]) @ wvT_head + bv_head
    since sum_l w = 1 after normalization.
  - Per-(n,h) softmax uses unnormalized exp (scores are O(0.2), no overflow)
    with a multiplicative 0/1 mask; normalization by the sum computed via an
    appended ones-column in the same matmul.
  - Backends with no tokens produce 0 exactly as the reference does (the
    reference falls back to the full mask but then zeroes by has_tokens).

Sharding: data-parallel over batch. 16 batches / 8 cores = 2 batches per core.
No collectives; outputs are disjoint.

Pipeline: per batch, L is split into pieces. PE runs scores two pieces ahead
of the y-accumulation so the cross-engine exp->mask-mult chain (ACT->DVE)
never stalls the y matmuls; exp/w are additionally produced in half-piece
tiles for finer dependencies.
"""

import os
import sys

sys.path.insert(0, "/opt/trn_rl_repo")

import numpy as np
import ml_dtypes

import concourse.bass as bass
import concourse.bacc as bacc
import concourse.tile as tile
from concourse import mybir
from concourse.bass_utils import run_bass_kernel_spmd

BF16 = ml_dtypes.bfloat16
FP8 = ml_dtypes.float8_e4m3
QW_SHIFT = 13  # qw values ~7e-4 underflow fp8; pre-scale by 2**13, undo in exp scale
F32 = np.float32

B, L, D = 16, 8192, 256
H, HD, NB = 8, 32, 8
NCORES = 8
BPC = B // NCORES          # batches per core
NCH = L // 128             # 64 l-chunks of 128
SCALE = 1.0 / np.sqrt(HD)
NH = NB * H                # 64 (n, h) pairs
JW = D + 1                 # 257: x chunk width incl. ones column

_CACHE = {}
LAST_RESULT = None


def _patched_act_tables():
    """Make the act-table chooser land Exp AND Ln in one set
    (natural_log_exp_and_others) instead of thrashing between
    exp_and_others and natural_log (1.28us reload per switch).
    Order/size of the dict is preserved so act_func_set_ids stay valid."""
    from concourse.hw_specs import get_activation_tables

    AF = mybir.ActivationFunctionType

    def patched(arch):
        t = {k: set(v) for k, v in get_activation_tables(arch).items()}
        for name in t:
            if name != "natural_log_exp_and_others":
                t[name].discard(AF.Exp)
                t[name].discard(AF.Ln)
        return t

    return patched


def _build_nc(zero_bv=True, zero_br=True, unit_ln=True):
    nc = bacc.Bacc("TRN2", target_bir_lowering=False)
    dt = mybir.dt

    PIECES = [16, 16, 8, 16, 8]   # big early pieces keep the DMA ring fed
    # whole value path in fp8 e3m4 (4 mantissa bits): sim rel err 1.35e-2 vs
    # the 2e-2 gate, and the xe stream halves vs bf16
    NP = len(PIECES)
    OFFS = [sum(PIECES[:k]) for k in range(NP + 1)]
    LOOK = 2                      # scores lookahead (pieces)

    xT_d = nc.dram_tensor("xT", [BPC, 2, 128, L], dt.float8e4, kind="ExternalInput")
    xe8_d = nc.dram_tensor("xe8", [BPC, 128, NCH * JW], dt.float8e3, kind="ExternalInput")
    m_d = nc.dram_tensor("m", [BPC, 128, NCH * NB], dt.bfloat16, kind="ExternalInput")
    qwT_d = nc.dram_tensor("qwT", [2, 128, H], dt.float8e4, kind="ExternalInput")
    wvT_d = nc.dram_tensor("wvT", [2, 128, D], dt.bfloat16, kind="ExternalInput")
    woT_d = nc.dram_tensor("woT", [2, 128, D], dt.bfloat16, kind="ExternalInput")
    if unit_ln:
        lht_d = nc.dram_tensor("lht", [BPC, NB, 1], dt.float32, kind="ExternalInput")
    else:
        gf_d = nc.dram_tensor("gf", [BPC, NB, D], dt.float32, kind="ExternalInput")
        bf_d = nc.dram_tensor("bf", [BPC, NB, D], dt.float32, kind="ExternalInput")
    br_d = nc.dram_tensor("br", [NB, D], dt.float32, kind="ExternalInput")
    bv_d = nc.dram_tensor("bv", [128, 2], dt.float32, kind="ExternalInput")
    id_d = nc.dram_tensor("ident", [NH, NH], dt.bfloat16, kind="ExternalInput")
    out_d = nc.dram_tensor("out", [BPC, NB, D], dt.float32, kind="ExternalOutput")

    AF = mybir.ActivationFunctionType

    with tile.TileContext(nc) as tc:
        with (
            tc.tile_pool(name="consts", bufs=1) as consts,
            tc.tile_pool(name="big", bufs=2) as big,
            tc.tile_pool(name="work", bufs=2) as work,
            tc.tile_pool(name="psc", bufs=3, space="PSUM") as psc,
            tc.tile_pool(name="psy", bufs=2, space="PSUM") as psy,
            tc.tile_pool(name="pst", bufs=2, space="PSUM") as pst,
        ):
            # qwT is on the scores critical path: load first
            qwT_sb = consts.tile([128, 2, H], dt.float8e4)
            nc.scalar.dma_start(out=qwT_sb, in_=qwT_d[:].rearrange("c p h -> p c h"))

            def load_tail_consts():
                # needed only from the first batch's tail onward
                wvT_sb = consts.tile([128, 2, D], dt.bfloat16)
                nc.gpsimd.dma_start(out=wvT_sb, in_=wvT_d[:].rearrange("c p e -> p c e"))
                woT_sb = consts.tile([128, 2, D], dt.bfloat16)
                nc.gpsimd.dma_start(out=woT_sb, in_=woT_d[:].rearrange("c p e -> p c e"))
                br_sb = None
                if not zero_br:
                    br_sb = consts.tile([NB, D], dt.float32)
                    nc.gpsimd.dma_start(out=br_sb, in_=br_d[:])
                bv_sb = None
                if not zero_bv:
                    bv_sb = consts.tile([128, 2], dt.float32)
                    nc.gpsimd.dma_start(out=bv_sb, in_=bv_d[:])
                id_sb = consts.tile([NH, NH], dt.bfloat16)
                nc.gpsimd.dma_start(out=id_sb, in_=id_d[:])
                eps_sb = consts.tile([128, 1], dt.float32)
                nc.vector.memset(eps_sb, 1e-5)
                return wvT_sb, woT_sb, br_sb, bv_sb, id_sb, eps_sb

            tail_consts = None
            for i in range(BPC):
                # ---- loads, interleaved in consumption order ----
                xT_sb, xe_sb = {}, {}
                m_sb = None

                def load_xe(qe):
                    # one DMA per piece (ring throughput is sensitive to DMA
                    # count); value path is e3m4 end to end
                    pce = PIECES[qe]
                    xe = big.tile([128, pce, JW], dt.float8e3, tag=f"xe{qe}_0")
                    nc.sync.dma_start(
                        out=xe,
                        in_=xe8_d[
                            i, :, OFFS[qe] * JW:OFFS[qe + 1] * JW
                        ].rearrange("p (c j) -> p c j", j=JW),
                    )
                    xe_sb[(qe, 0)] = xe
                ht_sb = g_sb = b_sb = None
                # xT pieces 0..LOOK first (scores critical path + lookahead)
                for q in range(NP):
                    pc = PIECES[q]
                    t = big.tile([128, 2, pc * 128], dt.float8e4, tag=f"xT{q}")
                    nc.sync.dma_start(
                        out=t,
                        in_=xT_d[i, :, :, OFFS[q] * 128:OFFS[q + 1] * 128].rearrange(
                            "c p l -> p c l"
                        ),
                    )
                    xT_sb[q] = t
                    if q == 0:
                        m_sb = work.tile([128, NCH, NB], dt.bfloat16, tag="m")
                        nc.scalar.dma_start(
                            out=m_sb, in_=m_d[i].rearrange("p (c n) -> p c n", n=NB)
                        )
                    if q >= LOOK:
                        qe = q - LOOK
                        load_xe(qe)
                    if i == 0 and q == LOOK:
                        tail_consts = load_tail_consts()
                for qe in range(NP - LOOK, NP):
                    load_xe(qe)
                if unit_ln:
                    lht_sb = work.tile([NB, 1], dt.float32, tag="lht")
                    nc.gpsimd.dma_start(out=lht_sb, in_=lht_d[i])
                else:
                    g_sb = work.tile([NB, D], dt.float32, tag="g")
                    nc.gpsimd.dma_start(out=g_sb, in_=gf_d[i])
                    b_sb = work.tile([NB, D], dt.float32, tag="b")
                    nc.gpsimd.dma_start(out=b_sb, in_=bf_d[i])
                wvT_sb, woT_sb, br_sb, bv_sb, id_sb, eps_sb = tail_consts

                # ---- pipelined pieces: scores -> exp/w (half tiles) -> y ----
                y_ps = psy.tile([NH, JW], dt.float32, tag="y")

                def scores_piece(q):
                    pc = PIECES[q]
                    sc_ps = psc.tile([128, pc * H], dt.float32, tag="sc")
                    for cl in range(pc):
                        for dc in range(2):
                            nc.tensor.matmul(
                                sc_ps[:, cl * 8:(cl + 1) * 8],
                                lhsT=xT_sb[q][:, dc, cl * 128:(cl + 1) * 128],
                                rhs=qwT_sb[:, dc, :],
                                start=(dc == 0),
                                stop=(dc == 1),
                            )
                    return sc_ps

                def expw_piece(q, sc_ps):
                    pc = PIECES[q]
                    h1 = pc // 2
                    ws = []
                    for part, (c0, c1) in enumerate([(0, h1), (h1, pc)]):
                        seg = c1 - c0
                        E = work.tile([128, seg, H], dt.bfloat16, tag=f"E{q}_{part}")
                        nc.scalar.activation(
                            out=E.rearrange("p c h -> p (c h)"),
                            in_=sc_ps[:, c0 * 8:c1 * 8],
                            func=AF.Exp,
                            scale=float(2.0 ** -QW_SHIFT),
                        )
                        w = big.tile([128, seg, NB, H], dt.bfloat16, tag=f"w{q}_{part}")
                        nc.vector.tensor_tensor(
                            out=w,
                            in0=E.unsqueeze(2).broadcast_to([128, seg, NB, H]),
                            in1=m_sb[:, OFFS[q] + c0:OFFS[q] + c1].unsqueeze(3).broadcast_to(
                                [128, seg, NB, H]
                            ),
                            op=mybir.AluOpType.mult,
                        )
                        ws.append((c0, seg, w))
                    return ws

                def y_piece(q, ws):
                    for c0, seg, w in ws:
                        for k in range(seg):
                            cl = c0 + k
                            nc.tensor.matmul(
                                y_ps,
                                lhsT=w[:, k],
                                rhs=xe_sb[(q, 0)][:, cl],
                                start=(q == 0 and cl == 0),
                                stop=(q == NP - 1 and cl == PIECES[q] - 1),
                            )

                wq = {}
                for q in range(NP):
                    wq[q] = expw_piece(q, scores_piece(q))
                    if q >= LOOK:
                        y_piece(q - LOOK, wq.pop(q - LOOK))
                for q in range(NP - LOOK, NP):
                    y_piece(q, wq.pop(q))

                # ---- normalize ----
                # host builds m with the reference's full-mask fallback for
                # empty backends, so s = sum(w) > 0 always: no epsilon needed
                r_sb = work.tile([NH, 1], dt.float32, tag="r")
                nc.vector.reciprocal(r_sb, y_ps[:, D:JW])
                yn_sb = work.tile([NH, D], dt.bfloat16, tag="yn")
                nc.vector.tensor_scalar_mul(yn_sb, y_ps[:, 0:D], r_sb)

                # ---- transpose y_norm -> yT [d, nh] (2 chunks) ----
                yT_sb = work.tile([128, 2, NH], dt.bfloat16, tag="yT")
                for ec in range(2):
                    tr_ps = pst.tile([128, NH], dt.bfloat16, tag="tail")
                    nc.tensor.transpose(
                        tr_ps, yn_sb[:, ec * 128:(ec + 1) * 128], id_sb
                    )
                    nc.vector.tensor_copy(yT_sb[:, ec, :], tr_ps)

                # ---- ctxT[e', (t, n)] via per-head matmuls ----
                cx_ps = pst.tile([128, 2 * NB], dt.float32, tag="tail")
                for h in range(H):
                    t, r4 = divmod(h, 4)
                    r0 = r4 * 32
                    for dc in range(2):
                        nc.tensor.matmul(
                            cx_ps[r0:r0 + 32, t * NB:(t + 1) * NB],
                            lhsT=wvT_sb[:, dc, 32 * h:32 * h + 32],
                            rhs=yT_sb[:, dc, :].rearrange("p (n h2) -> p n h2", h2=H)[:, :, h],
                            start=(dc == 0),
                            stop=(dc == 1),
                            tile_position=(0, r0),
                        )
                cx_sb = work.tile([128, 2 * NB], dt.bfloat16, tag="cxs")
                nc.vector.tensor_copy(cx_sb, cx_ps)
                if not zero_bv:
                    for t in range(2):
                        nc.vector.tensor_scalar_add(
                            cx_sb[:, t * NB:(t + 1) * NB],
                            cx_sb[:, t * NB:(t + 1) * NB],
                            bv_sb[:, t:t + 1],
                        )

                # ---- out_proj + bias + LayerNorm (folded has_tokens) ----
                op_ps = pst.tile([NB, D], dt.float32, tag="tail")
                for t in range(2):
                    nc.tensor.matmul(
                        op_ps,
                        lhsT=cx_sb[:, t * NB:(t + 1) * NB],
                        rhs=woT_sb[:, t, :],
                        start=(t == 0),
                        stop=(t == 1),
                    )
                if zero_br:
                    o_ap = op_ps
                else:
                    o_sb = work.tile([NB, D], dt.float32, tag="o")
                    nc.vector.tensor_tensor(o_sb, op_ps, br_sb, op=mybir.AluOpType.add)
                    o_ap = o_sb
                st_sb = work.tile([NB, 6], dt.float32, tag="st")
                nc.vector.bn_stats(st_sb, o_ap)
                mv_sb = work.tile([NB, 2], dt.float32, tag="mv")
                nc.vector.bn_aggr(mv_sb, st_sb)
                lnv_sb = work.tile([NB, 1], dt.float32, tag="lnv")
                nc.scalar.activation(
                    lnv_sb, mv_sb[:, 1:2], func=AF.Ln,
                    bias=eps_sb[0:NB],
                )
                rstd_sb = work.tile([NB, 1], dt.float32, tag="rstd")
                if unit_ln:
                    # rstd = exp(-0.5*ln(var+eps) + log(has_tokens)): the
                    # -1e30 bias for empty backends drives rstd to exactly 0,
                    # folding the has_tokens gate into this activation
                    nc.scalar.activation(
                        rstd_sb, lnv_sb, func=AF.Exp,
                        scale=-0.5, bias=lht_sb,
                    )
                else:
                    nc.scalar.activation(
                        rstd_sb, lnv_sb, func=AF.Exp,
                        scale=-0.5,
                    )
                c_sb = work.tile([NB, D], dt.float32, tag="c")
                if unit_ln:
                    nc.vector.tensor_scalar(
                        c_sb, o_ap, mv_sb[:, 0:1], rstd_sb,
                        op0=mybir.AluOpType.subtract, op1=mybir.AluOpType.mult,
                    )
                else:
                    nc.vector.tensor_scalar(
                        c_sb, o_ap, mv_sb[:, 0:1], rstd_sb,
                        op0=mybir.AluOpType.subtract, op1=mybir.AluOpType.mult,
                    )
                    nc.vector.tensor_tensor(c_sb, c_sb, g_sb, op=mybir.AluOpType.mult)
                    nc.vector.tensor_tensor(c_sb, c_sb, b_sb, op=mybir.AluOpType.add)

                # SWDGE store for batch 0 keeps the result-gated store off the
                # Sync HWDGE ring (later loads would queue behind it); the final
                # batch has no later loads, so use the faster HWDGE path.
                if i == BPC - 1:
                    nc.sync.dma_start(out=out_d[i], in_=c_sb)
                else:
                    nc.gpsimd.dma_start(out=out_d[i], in_=c_sb)

    import concourse.bacc as bacc_mod

    orig_tables = bacc_mod.get_activation_tables
    bacc_mod.get_activation_tables = _patched_act_tables()
    try:
        nc.compile()
    finally:
        bacc_mod.get_activation_tables = orig_tables
    return nc


def _get_nc(zero_bv=True, zero_br=True, unit_ln=True):
    key = ("nc", zero_bv, zero_br, unit_ln)
    if key not in _CACHE:
        _CACHE[key] = _build_nc(zero_bv, zero_br, unit_ln)
    return _CACHE[key]


def _prep(inputs, unit_ln):
    x = np.asarray(inputs["x"], F32)
    query = np.asarray(inputs["query"], F32)
    ipw = np.asarray(inputs["in_proj_weight"], F32)
    ipb = np.asarray(inputs["in_proj_bias"], F32)
    opw = np.asarray(inputs["out_proj_weight"], F32)
    opb = np.asarray(inputs["out_proj_bias"], F32)
    gamma = np.asarray(inputs["ln_gamma"], F32)
    beta = np.asarray(inputs["ln_beta"], F32)
    mask = np.asarray(inputs["mask"]).astype(bool)
    bid = np.asarray(inputs["backend_id"]).astype(np.int32)
    nbm = int(np.asarray(inputs["n_backends_max"]))
    assert nbm == NB and x.shape == (B, L, D)

    wq, wk, wv = ipw[0:D], ipw[D:2 * D], ipw[2 * D:3 * D]
    bq, bk, bv = ipb[0:D], ipb[D:2 * D], ipb[2 * D:3 * D]

    qv = query[0, 0] @ wq.T + bq                      # (256,)
    qh = qv.reshape(H, HD)
    qw = np.einsum("hj,hjd->hd", qh, wk.reshape(H, HD, D)) * SCALE  # (8, 256)
    # bk contribution is constant per h -> cancels in softmax normalization.

    qwT = np.ascontiguousarray(qw.T * 2.0 ** QW_SHIFT).reshape(2, 128, H).astype(FP8)
    wvT = np.ascontiguousarray(wv.T).reshape(2, 128, D).astype(BF16)
    woT = np.ascontiguousarray(opw.T).reshape(2, 128, D).astype(BF16)
    bv_dev = np.ascontiguousarray(bv.reshape(2, 128).T).astype(F32)   # [e', t]
    br = np.ascontiguousarray(np.broadcast_to(opb, (NB, D))).astype(F32)
    ident = np.eye(NH, dtype=BF16)

    bm = mask[:, :, None] & (bid[:, :, None] == np.arange(NB, dtype=np.int32))
    ht_b = bm.any(1)                                  # (16, 8) bool
    ht = ht_b.astype(F32)
    # reference's fallback: empty backends use the full mask (their output is
    # zeroed by has_tokens anyway); guarantees sum(w) > 0 on device. If even
    # the full mask is empty, use all-ones (still zeroed by ht).
    fall = np.where(mask.any(axis=1)[:, None], mask, True)
    bm_safe = np.where(ht_b[:, None, :], bm, fall[:, :, None])
    m_host = np.ascontiguousarray(
        bm_safe.reshape(B, NCH, 128, NB).transpose(0, 2, 1, 3)
    ).astype(BF16).reshape(B, 128, NCH * NB)

    x_bf = x.astype(BF16)
    xT = np.ascontiguousarray(x.transpose(0, 2, 1)).reshape(B, 2, 128, L).astype(FP8)
    x_pcd = x_bf.reshape(B, NCH, 128, D).transpose(0, 2, 1, 3)  # [B,128,NCH,D]
    E3 = ml_dtypes.float8_e3m4
    xe8 = np.empty((B, 128, NCH, JW), E3)
    xe8[..., :D] = x_pcd
    xe8[..., D] = 1
    xe8 = xe8.reshape(B, 128, NCH * JW)

    in_maps = []
    for c in range(NCORES):
        sl = slice(BPC * c, BPC * (c + 1))
        im = {
            "xT": xT[sl], "xe8": xe8[sl], "m": m_host[sl],
            "qwT": qwT, "wvT": wvT, "woT": woT,
            "br": br, "bv": bv_dev,
            "ident": ident,
        }
        if unit_ln:
            lht = np.where(ht[sl] > 0, 0.0, -1e30).astype(F32)
            im["lht"] = np.ascontiguousarray(lht[:, :, None])
        else:
            gf = np.ascontiguousarray(gamma[None, None, :] * ht[sl, :, None]).astype(F32)
            bf = np.ascontiguousarray(beta[None, None, :] * ht[sl, :, None]).astype(F32)
            im["gf"] = gf
            im["bf"] = bf
        in_maps.append(im)
    return in_maps


def kernel(**inputs):
    global LAST_RESULT
    gamma = np.asarray(inputs["ln_gamma"], F32)
    beta = np.asarray(inputs["ln_beta"], F32)
    unit_ln = bool((gamma == 1.0).all() and not beta.any())
    in_maps = _prep(inputs, unit_ln)
    ipb = np.asarray(inputs["in_proj_bias"], F32)
    opb = np.asarray(inputs["out_proj_bias"], F32)
    nc = _get_nc(zero_bv=not ipb[2 * D:].any(), zero_br=not opb.any(),
                 unit_ln=unit_ln)
    res = run_bass_kernel_spmd(nc, in_maps, list(range(NCORES)))
    LAST_RESULT = res
    out = np.concatenate([res.results[c]["out"] for c in range(NCORES)], axis=0)
    return np.ascontiguousarray(out.astype(F32))


if __name__ == "__main__":
    nc = _get_nc()
    print("traced ok:", nc)
